# revision 1
# baseline (speedup 1.0000x reference)
"""GAT (GATConv + Linear) Trainium2 kernel, 8-core edge-parallel.

Strategy
--------
Edges (incl. self-loops) are sorted by dst and partitioned across the 8
cores by dst range (each core owns N/8 destination nodes), so the
segment-softmax and the scatter-add are fully core-local (no collective).

The host does the "gather": for each core it materializes the per-edge
source features x[src] already transposed into matmul-ready [K=feat,
M=edge] tiles. On device, per 128-edge chunk:
  h_e | a_src_e = xeT_chunk.T @ [W_gat | W_gat@att_src]   (PE, fp32 PSUM)
  a_dst_e      += expand_onehot.T @ ad_group              (PE, into the
                  same PSUM columns, so logits come out pre-summed)
  ex = exp(leaky_relu(logits))                            (ACT)
  msg = [ex*h_e | ex]                                     (DVE broadcast)
  OUT_group += scatter_onehot.T @ msg                     (PE, PSUM accum)
Group finalize: alpha-normalize by the denominator columns, +bias, relu,
transpose (DVE), @W_lin + b_lin, DMA out.

Max-subtraction in the softmax is skipped: logits here are O(+-6), well
within fp32 exp range, and the result is mathematically identical.
"""

import os
import sys
import math
import numpy as np

sys.path.insert(0, "/opt/trn_rl_repo")

NC_CORES = 8
SUP = 16   # chunks per DMA slab
BLK = 2    # chunks per logit/exp batch block
BCB = 4    # chunks per dl-broadcast / expand-one-hot batch
PAD_DL = 999.0
LAST_RESULTS = None  # BassKernelResults of the most recent HW run
LAST_WALL_S = None   # min wall seconds of a warm run (BASS_GAT_TIME mode)
LAST_SCHED_NS = None  # tile scheduler cost-model predicted makespan

F32 = None  # set after mybir import


def _ceil_div(a, b):
    return (a + b - 1) // b


def _preprocess(x, edge_index, W_gat, att_src, att_dst, bias_gat, W_lin, b_lin):
    """Returns (per_core_inputs, meta) for the SPMD program."""
    N, IN = x.shape
    H, C = att_src.shape[1], att_src.shape[2]
    OUT = W_lin.shape[1]
    E = edge_index.shape[1]

    x = np.asarray(x, np.float32)
    W_gat = np.asarray(W_gat, np.float32)
    att_src = np.asarray(att_src, np.float32).reshape(H, C)
    att_dst = np.asarray(att_dst, np.float32).reshape(H, C)
    bias_gat = np.asarray(bias_gat, np.float32)
    W_lin = np.asarray(W_lin, np.float32)
    b_lin = np.asarray(b_lin, np.float32)

    # fold attention vectors into weight-space projections
    # a_src[n,h] = sum_c (x@W)[n,h*C+c]*att_src[h,c] = x @ V_src[:, h]
    Wr = W_gat.reshape(IN, H, C)
    V_src = np.einsum("ihc,hc->ih", Wr, att_src).astype(np.float32)  # [IN, H]
    V_dst = np.einsum("ihc,hc->ih", Wr, att_dst).astype(np.float32)  # [IN, H]
    W_aug = np.concatenate([W_gat, V_src], axis=1)  # [IN, 256+H]
    WA = IN + H  # 264

    # edges + self loops, sorted by dst
    src = np.concatenate([edge_index[0], np.arange(N)]).astype(np.int64)
    dst = np.concatenate([edge_index[1], np.arange(N)]).astype(np.int64)
    order = np.argsort(dst, kind="stable")
    src_s = src[order].astype(np.int32)
    dst_s = dst[order].astype(np.int32)

    ndst = _ceil_div(N, NC_CORES)            # dst nodes per core
    G = _ceil_div(ndst, 128)                 # dst groups of 128 per core
    KIN = _ceil_div(IN, 128)                 # k-chunks (2)

    # group edge ranges for every (core, group)
    lo = np.empty((NC_CORES, G), np.int64)
    hi = np.empty((NC_CORES, G), np.int64)
    for d in range(NC_CORES):
        base = d * ndst
        for g in range(G):
            a = base + g * 128
            b = min(base + (g + 1) * 128, min((d + 1) * ndst, N))
            lo[d, g] = np.searchsorted(dst_s, a)
            hi[d, g] = np.searchsorted(dst_s, max(a, b))
    cnt = (hi - lo).astype(np.int64)
    K = max(1, int(_ceil_div(cnt.max(), 128)))  # chunks per group (uniform)
    NCHUNK = G * K
    NSUP = _ceil_div(NCHUNK, SUP)

    per_core = []
    for d in range(NC_CORES):
        srcs = np.zeros(NCHUNK * 128, np.int64)
        dls = np.full(NCHUNK * 128, PAD_DL, np.float32)
        for g in range(G):
            a, b = lo[d, g], hi[d, g]
            n = b - a
            s0 = g * K * 128
            srcs[s0:s0 + n] = src_s[a:b]
            dls[s0:s0 + n] = (dst_s[a:b] - (d * ndst + g * 128)).astype(np.float32)

        # per-edge transposed features, p-major for contiguous DMA slabs:
        # xeT[p, c, k, e] = x[src[c*128+e], k*128+p]
        xe = x[srcs]                                    # [NCHUNK*128, IN]
        xe = xe.reshape(NCHUNK, 128, KIN, 128)          # [c, e, k, p]
        xeT = np.ascontiguousarray(xe.transpose(3, 0, 2, 1))  # [p, c, k, e]

        # dst-local index tables
        dlT = np.ascontiguousarray(
            dls.reshape(NCHUNK, 128).T)                 # [128, NCHUNK]
        dlR = dls.copy()                                # [NCHUNK*128]

        # own dst nodes' features, transposed (for a_dst):
        dn = np.arange(G * 128, dtype=np.int64) + d * ndst
        dn = np.clip(dn, 0, N - 1)
        xd = x[dn].reshape(G, 128, KIN, 128)            # [g, n, k, p]
        xdT = np.ascontiguousarray(xd.transpose(3, 0, 2, 1))  # [p, g, k, n]

        per_core.append({"xeT": xeT, "xdT": xdT, "dlT": dlT, "dlR": dlR})

    # constants blob [128, CC]
    cols = {}
    parts = []
    cc = 0

    def add(name, arr):
        nonlocal cc
        arr = np.asarray(arr, np.float32)
        assert arr.shape[0] == 128
        cols[name] = cc
        parts.append(arr)
        cc += arr.shape[1]

    add("iota_col", np.arange(128, dtype=np.float32)[:, None])       # value=p
    add("eps", np.full((128, 1), 1e-16, np.float32))
    add("c02", np.full((128, 1), 0.2, np.float32))
    add("ident", np.eye(128, dtype=np.float32))
    ones = np.zeros((128, 128), np.float32)
    ones[0, :] = 1.0
    add("ones", ones)                                                # row0=1
    add("iota_fr", np.broadcast_to(
        np.arange(128, dtype=np.float32), (128, 128)).copy())        # value=d
    wa = W_aug.reshape(KIN, 128, WA).transpose(1, 0, 2).reshape(128, KIN * WA)
    add("w_aug", wa)                                                 # [p,(k,col)]
    vd = V_dst.reshape(KIN, 128, H).transpose(1, 0, 2).reshape(128, KIN * H)
    add("v_dst", vd)
    wl = W_lin.reshape(KIN, 128, OUT).transpose(1, 0, 2).reshape(128, KIN * OUT)
    add("w_lin", wl)
    add("b_lin", np.broadcast_to(b_lin, (128, OUT)).copy())
    add("bias_gat", np.broadcast_to(bias_gat, (128, IN)).copy())
    cst = np.concatenate(parts, axis=1)

    meta = dict(N=N, IN=IN, H=H, C=C, OUT=OUT, WA=WA, KIN=KIN,
                ndst=ndst, G=G, K=K, NCHUNK=NCHUNK, NSUP=NSUP,
                cols=cols, CC=cc)
    return per_core, cst, meta


def _build_program(meta):
    import concourse.bass as bass
    import concourse.mybir as mybir
    import concourse.tile as tile
    from concourse import bacc
    import concourse.bass_interp as _bi

    # capture the tile scheduler's simulated makespan (cost-model prediction)
    _clk = []
    _orig_sim = _bi.CoreSim.simulate

    def _sim_patch(self, *a, **k):
        r = _orig_sim(self, *a, **k)
        try:
            _clk.append(self.time)
        except Exception:
            pass
        return r

    _bi.CoreSim.simulate = _sim_patch

    f32 = mybir.dt.float32
    G, K, NCHUNK = meta["G"], meta["K"], meta["NCHUNK"]
    KIN, WA, H, OUT, IN = meta["KIN"], meta["WA"], meta["H"], meta["OUT"], meta["IN"]
    CC, cols = meta["CC"], meta["cols"]
    C = meta["C"]

    nc = bacc.Bacc()
    xeT_in = nc.dram_tensor("xeT", [128, NCHUNK, KIN, 128], f32, kind="ExternalInput")
    xdT_in = nc.dram_tensor("xdT", [128, G, KIN, 128], f32, kind="ExternalInput")
    dlT_in = nc.dram_tensor("dlT", [128, NCHUNK], f32, kind="ExternalInput")
    dlR_in = nc.dram_tensor("dlR", [NCHUNK * 128], f32, kind="ExternalInput")
    cst_in = nc.dram_tensor("cst", [128, CC], f32, kind="ExternalInput")
    out_t = nc.dram_tensor("out", [G * 128, OUT], f32, kind="ExternalOutput")

    EQ = mybir.AluOpType.is_equal
    MUL = mybir.AluOpType.mult
    ADD = mybir.AluOpType.add
    AF = mybir.ActivationFunctionType

    with tile.TileContext(nc) as tc:
        with tc.tile_pool(name="cpool", bufs=1) as cpool:
            cst = cpool.tile([128, CC], f32)
            nc.sync.dma_start(out=cst[:], in_=cst_in[:])
            ad_loc = cpool.tile([128, G, H], f32)

            def cs(name, w):
                return cst[:, cols[name]:cols[name] + w]

            # ---- phase 0: a_dst for this core's dst nodes ----
            with tc.tile_pool(name="p0", bufs=1) as p0, \
                 tc.tile_pool(name="p0ps", bufs=2, space="PSUM") as p0ps:
                xd_sb = p0.tile([128, G, KIN, 128], f32)
                nc.sync.dma_start(out=xd_sb[:], in_=xdT_in[:])
                for g in range(G):
                    ad_ps = p0ps.tile([128, H], f32, space="PSUM")
                    for k in range(KIN):
                        nc.tensor.matmul(
                            ad_ps[:], xd_sb[:, g, k, :],
                            cst[:, cols["v_dst"] + k * H: cols["v_dst"] + (k + 1) * H],
                            start=(k == 0), stop=(k == KIN - 1))
                    nc.vector.tensor_copy(out=ad_loc[:, g, :], in_=ad_ps[:])

            # ---- main edge loop ----
            with tc.tile_pool(name="slab", bufs=4) as slab_pool, \
                 tc.tile_pool(name="wrk", bufs=5) as wrk, \
                 tc.tile_pool(name="grp", bufs=2) as grp, \
                 tc.tile_pool(name="psh", bufs=2 * BLK, space="PSUM") as psh, \
                 tc.tile_pool(name="pso", bufs=2, space="PSUM") as pso, \
                 tc.tile_pool(name="psb", bufs=1, space="PSUM") as psb, \
                 tc.tile_pool(name="psf", bufs=1, space="PSUM") as psf:

                xeT_sb = None
                dlT_sb = None
                dlR_sb = None
                soh_sb = None
                bc_ps = None
                out_ps = None
                h_blk = [None] * BLK
                for c0 in range(0, NCHUNK, BLK):
                    blkc = min(BLK, NCHUNK - c0)
                    # ---- phase A: per-edge h + logits for the block ----
                    for b2 in range(blkc):
                        c = c0 + b2
                        s, b = divmod(c, SUP)
                        if b == 0:
                            supc = min(SUP, NCHUNK - s * SUP)
                            xeT_sb = slab_pool.tile([128, SUP, KIN, 128], f32,
                                                    tag="xeT")
                            nc.sync.dma_start(
                                out=xeT_sb[:, :supc, :, :],
                                in_=xeT_in[:, s * SUP:s * SUP + supc, :, :])
                            dlT_sb = slab_pool.tile([128, SUP], f32, tag="dlT")
                            nc.sync.dma_start(
                                out=dlT_sb[:, :supc],
                                in_=dlT_in[:, s * SUP:s * SUP + supc])
                            dlR_sb = slab_pool.tile([1, SUP * 128], f32, tag="dlR")
                            nc.sync.dma_start(
                                out=dlR_sb[:, :supc * 128],
                                in_=dlR_in[s * SUP * 128:(s * SUP + supc) * 128])
                            # batched scatter one-hots for the whole slab:
                            # soh[e, (b, d)] = (dl[b, e] == d)
                            soh_sb = slab_pool.tile([128, SUP, 128], f32, tag="soh")
                            nc.vector.tensor_tensor(
                                out=soh_sb[:, :supc, :],
                                in0=dlT_sb[:, :supc].to_broadcast(
                                    [128, supc, 128]),
                                in1=cs("iota_fr", 128)[:, None, :].to_broadcast(
                                    [128, supc, 128]),
                                op=EQ)
                        if b2 == 0 and c0 % BCB == 0:
                            # dl broadcast + expand one-hots for BCB chunks:
                            # eoh[d, (q, e)] = (d == dl[q, e])
                            bcc = min(BCB, NCHUNK - c0)
                            bc_ps = psb.tile([128, BCB * 128], f32, space="PSUM")
                            ones_row = cst[0:1, cols["ones"]:cols["ones"] + 128]
                            boff = (c0 % SUP) * 128
                            nc.tensor.matmul(
                                bc_ps[:, :bcc * 128], ones_row,
                                dlR_sb[:, boff:boff + bcc * 128],
                                start=True, stop=True)
                            eohb = wrk.tile([128, BCB * 128], f32, tag="eohb")
                            nc.vector.tensor_tensor(
                                out=eohb[:, :bcc * 128],
                                in0=cs("iota_col", 1).to_broadcast([128, bcc * 128]),
                                in1=bc_ps[:, :bcc * 128], op=EQ)
                        g = c // K
                        h_ps = psh.tile([128, WA], f32, space="PSUM", tag="h")
                        h_blk[b2] = h_ps
                        for k in range(KIN):
                            nc.tensor.matmul(
                                h_ps[:], xeT_sb[:, b, k, :],
                                cst[:, cols["w_aug"] + k * WA: cols["w_aug"] + (k + 1) * WA],
                                start=(k == 0), stop=False)

                    for b2 in range(blkc):
                        c = c0 + b2
                        g = c // K
                        q = c % BCB
                        nc.tensor.matmul(
                            h_blk[b2][:, IN:IN + H],
                            eohb[:, q * 128:(q + 1) * 128], ad_loc[:, g, :],
                            start=False, stop=True)

                    # ---- lrelu (DVE) into batch tile, one batched exp (ACT),
                    #      written straight into the msg block tile ----
                    msgb = wrk.tile([128, BLK, WA], f32, tag="msgb")
                    lrb = wrk.tile([128, BLK * H], f32, tag="lrb")
                    for b2 in range(blkc):
                        h_ps = h_blk[b2]
                        t02 = wrk.tile([128, H], f32, tag="t02")
                        nc.vector.tensor_tensor(
                            out=t02[:], in0=h_ps[:, IN:IN + H],
                            in1=cs("c02", 1).to_broadcast([128, H]), op=MUL)
                        nc.vector.tensor_tensor(
                            out=lrb[:, b2 * H:(b2 + 1) * H], in0=t02[:],
                            in1=h_ps[:, IN:IN + H], op=mybir.AluOpType.max)
                    nc.scalar.activation(
                        msgb[:, :blkc, IN:IN + H],
                        lrb[:, :blkc * H].rearrange("p (b h) -> p b h", b=blkc),
                        AF.Exp)

                    # ---- phase C: msg + scatter per chunk ----
                    for b2 in range(blkc):
                        c = c0 + b2
                        s, b = divmod(c, SUP)
                        g, i = divmod(c, K)
                        h_ps = h_blk[b2]
                        nc.vector.tensor_tensor(
                            out=msgb[:, b2, 0:IN].rearrange("p (h c) -> p h c", h=H),
                            in0=h_ps[:, 0:IN].rearrange("p (h c) -> p h c", h=H),
                            in1=msgb[:, b2, IN:IN + H][:, :, None]
                                .to_broadcast([128, H, C]),
                            op=MUL)
                        if i == 0:
                            out_ps = pso.tile([128, WA], f32, space="PSUM")
                        nc.tensor.matmul(out_ps[:], soh_sb[:, b, :], msgb[:, b2, :],
                                         start=(i == 0), stop=(i == K - 1))

                        if i != K - 1:
                            continue
                        # ---- group finalize ----
                        den = grp.tile([128, H], f32, tag="den")
                        nc.vector.tensor_tensor(
                            out=den[:], in0=out_ps[:, IN:IN + H],
                            in1=cs("eps", 1).to_broadcast([128, H]), op=ADD)
                        rec = grp.tile([128, H, 1], f32, tag="rec")
                        nc.vector.reciprocal(rec[:, :, 0], den[:])
                        gat = grp.tile([128, IN], f32, tag="gat")
                        nc.vector.tensor_tensor(
                            out=gat[:].rearrange("p (h c) -> p h c", h=H),
                            in0=out_ps[:, 0:IN].rearrange("p (h c) -> p h c", h=H),
                            in1=rec[:].to_broadcast([128, H, C]), op=MUL)
                        gatb = grp.tile([128, IN], f32, tag="gatb")
                        nc.vector.tensor_tensor(
                            out=gatb[:], in0=gat[:], in1=cs("bias_gat", IN), op=ADD)
                        gr = grp.tile([128, IN], f32, tag="gr")
                        nc.scalar.activation(gr[:], gatb[:], AF.Relu)
                        gatT = grp.tile([128, IN], f32, tag="gatT")
                        for k in range(KIN):
                            tr_ps = psf.tile([128, 128], f32, space="PSUM",
                                             tag="fin")
                            nc.tensor.transpose(out=tr_ps[:],
                                                in_=gr[:, k * 128:(k + 1) * 128],
                                                identity=cs("ident", 128))
                            nc.vector.tensor_copy(out=gatT[:, k * 128:(k + 1) * 128],
                                                  in_=tr_ps[:])
                        o_ps = psf.tile([128, OUT], f32, space="PSUM", tag="fin")
                        for k in range(KIN):
                            nc.tensor.matmul(
                                o_ps[:], gatT[:, k * 128:(k + 1) * 128],
                                cst[:, cols["w_lin"] + k * OUT: cols["w_lin"] + (k + 1) * OUT],
                                start=(k == 0), stop=(k == KIN - 1))
                        o_sb = grp.tile([128, OUT], f32, tag="o_sb")
                        nc.vector.tensor_tensor(
                            out=o_sb[:], in0=o_ps[:], in1=cs("b_lin", OUT), op=ADD)
                        nc.sync.dma_start(out=out_t[g * 128:(g + 1) * 128, :],
                                          in_=o_sb[:])

    _bi.CoreSim.simulate = _orig_sim
    global LAST_SCHED_NS
    LAST_SCHED_NS = int(max(_clk)) if _clk else None

    nc.finalize()
    return nc


def _timed_run(nc, in_maps, iters=8):
    """Mirror bass2jax.run_bass_via_pjrt but keep inputs device-resident and
    time warm repeated executions. Returns (results_core0_outs, min_wall_s)."""
    import time as _time
    import jax
    import numpy as _np
    from jax.sharding import Mesh, PartitionSpec, NamedSharding
    from jax.experimental.shard_map import shard_map
    import concourse.mybir as mybir
    from concourse import bass2jax

    bass2jax.install_neuronx_cc_hook()
    n_cores = len(in_maps)

    if nc.dbg_addr is not None:
        in_maps = [{**m, nc.dbg_addr.name: _np.zeros((1, 2), _np.uint32)}
                   for m in in_maps]
    partition_name = (nc.partition_id_tensor.name
                      if nc.partition_id_tensor else None)

    in_names, out_names, out_avals, zero_outs = [], [], [], []
    for alloc in nc.m.functions[0].allocations:
        if not isinstance(alloc, mybir.MemoryLocationSet):
            continue
        name = alloc.memorylocations[0].name
        if alloc.kind == "ExternalInput":
            if name == partition_name:
                continue
            in_names.append(name)
        elif alloc.kind == "ExternalOutput":
            out_names.append(name)
            dt = mybir.dt.np(alloc.dtype)
            out_avals.append(jax.core.ShapedArray(tuple(alloc.tensor_shape), dt))
            zero_outs.append(_np.zeros(tuple(alloc.tensor_shape), dt))
    n_params = len(in_names)
    all_in_names = in_names + out_names
    if partition_name is not None:
        all_in_names = all_in_names + [partition_name]

    def _body(*args):
        operands = list(args)
        if partition_name is not None:
            operands.append(bass2jax.partition_id_tensor())
        outs = bass2jax._bass_exec_p.bind(
            *operands,
            out_avals=tuple(out_avals),
            in_names=tuple(all_in_names),
            out_names=tuple(out_names),
            lowering_input_output_aliases=(),
            sim_require_finite=True,
            sim_require_nnan=True,
            nc=nc,
        )
        return tuple(outs)

    devices = jax.devices()[:n_cores]
    mesh = Mesh(_np.asarray(devices), ("core",))
    spec = PartitionSpec("core")
    sharded = jax.jit(shard_map(_body, mesh=mesh,
                                in_specs=(spec,) * (n_params + len(out_names)),
                                out_specs=(spec,) * len(out_names),
                                check_rep=False), keep_unused=True)
    sh = NamedSharding(mesh, spec)
    dev_args = [jax.device_put(
        _np.concatenate([_np.asarray(in_maps[c][nm]) for c in range(n_cores)], axis=0),
        sh) for nm in in_names]
    dev_zero = [jax.device_put(
        _np.zeros((n_cores * z.shape[0], *z.shape[1:]), z.dtype), sh)
        for z in zero_outs]

    out = sharded(*dev_args, *dev_zero)
    jax.block_until_ready(out)
    best = float("inf")
    for _ in range(iters):
        t0 = _time.perf_counter()
        out = sharded(*dev_args, *dev_zero)
        jax.block_until_ready(out)
        best = min(best, _time.perf_counter() - t0)
    outs = [_np.asarray(out[i]).reshape(n_cores, *out_avals[i].shape)
            for i in range(len(out_names))]
    per_core = [{nm: outs[i][c] for i, nm in enumerate(out_names)}
                for c in range(n_cores)]
    return per_core, best


def kernel(**inputs) -> np.ndarray:
    x = np.asarray(inputs["x"], np.float32)
    edge_index = np.asarray(inputs["edge_index"])
    N = x.shape[0]
    OUT = np.asarray(inputs["W_lin"]).shape[1]

    per_core, cst, meta = _preprocess(
        x, edge_index, inputs["W_gat"], inputs["att_src"], inputs["att_dst"],
        inputs["bias_gat"], inputs["W_lin"], inputs["b_lin"])

    nc = _build_program(meta)

    in_maps = []
    for d in range(NC_CORES):
        pc = per_core[d]
        in_maps.append({
            "xeT": pc["xeT"].reshape(128, meta["NCHUNK"], meta["KIN"], 128),
            "xdT": pc["xdT"],
            "dlT": pc["dlT"],
            "dlR": pc["dlR"],
            "cst": cst,
        })

    if os.environ.get("BASS_GAT_SIM"):
        from concourse import bass_interp
        outs = []
        for d in range(NC_CORES):
            sim = bass_interp.CoreSim(nc)
            for k, v in in_maps[d].items():
                sim.tensor(k)[:] = v
            sim.simulate()
            outs.append(np.array(sim.tensor("out")))
    elif os.environ.get("BASS_GAT_TIME"):
        global LAST_WALL_S
        per_core, LAST_WALL_S = _timed_run(nc, in_maps,
                                           iters=int(os.environ.get("BASS_GAT_TIME")))
        outs = [per_core[d]["out"] for d in range(NC_CORES)]
    else:
        from concourse.bass_utils import run_bass_kernel_spmd
        res = run_bass_kernel_spmd(nc, in_maps, core_ids=list(range(NC_CORES)))
        global LAST_RESULTS
        LAST_RESULTS = res
        outs = [res.results[d]["out"] for d in range(NC_CORES)]

    ndst = meta["ndst"]
    full = np.empty((N, OUT), np.float32)
    for d in range(NC_CORES):
        a = d * ndst
        b = min((d + 1) * ndst, N)
        full[a:b] = outs[d][0:b - a]
    return full

